# revision 2
# baseline (speedup 1.0000x reference)
"""BOXLoss Trainium2 kernel: 8-core data-parallel over N.

Shards the N=1M locations over 8 NeuronCores (padded to 125056 rows/core =
977*128). Three on-device phases separated by tiny AllReduce-max collectives:
  A: conf_g = max_i(mask*scores), cnt_i = sum_g mask
  B: raw = exp(conf_g*ln(s_i))*mask*iou  (stored bf16), mx_g = max_i raw
  C: wc = clip((raw+c)/(mx+c)), W1_i = sum_g mask*wc, W2_i = sum_g mask*wc^2,
     per-row loss terms; per-partition sums returned, final sum on host.
"""

import sys

sys.path.insert(0, "/opt/trn_rl_repo")

import numpy as np

from concourse import bacc, bass, mybir, tile
from concourse.bass_utils import run_bass_kernel_spmd

N = 1_000_000
G = 32
P = 128
NCORES = 8
NT = 977                      # free columns per partition per core
NC_ROWS = NT * P              # 125056 rows per core
NPAD = NC_ROWS * NCORES       # 1000448
CLAMP = 1e-4
CH = 32                       # n-chunk size (free-dim tile = CH*G = 1024)

f32 = mybir.dt.float32
bf16 = mybir.dt.bfloat16
i32 = mybir.dt.int32
Alu = mybir.AluOpType
Act = mybir.ActivationFunctionType
AX = mybir.AxisListType

_CACHE = {}


def _chunks():
    out = []
    n0 = 0
    while n0 < NT:
        k = min(CH, NT - n0)
        out.append((n0, k))
        n0 += k
    return out


def _build_program():
    nc = bacc.Bacc(
        "TRN2",
        target_bir_lowering=False,
        debug=False,
        enable_asserts=True,
        num_devices=NCORES,
    )
    iib = nc.dram_tensor("iib", [NC_ROWS, G], i32, kind="ExternalInput")
    lgt = nc.dram_tensor("lgt", [P, NT], f32, kind="ExternalInput")
    sco = nc.dram_tensor("sco", [P, NT], f32, kind="ExternalInput")
    iou = nc.dram_tensor("iou", [P, NT], f32, kind="ExternalInput")
    outp = nc.dram_tensor("outp", [P, 1], f32, kind="ExternalOutput")

    iib3 = iib.ap().rearrange("(n p) g -> p n g", p=P)
    chunks = _chunks()
    nch = len(chunks)
    rg = [list(range(NCORES))]

    with tile.TileContext(nc) as tc:
        with (
            tc.tile_pool(name="persist", bufs=1) as pp,
            tc.tile_pool(name="rows", bufs=1) as rp,
            tc.tile_pool(name="rowtmp", bufs=8) as rt,
            tc.tile_pool(name="stage", bufs=2) as sp,
            tc.tile_pool(name="dram", bufs=1, space="DRAM") as dp,
        ):
            B1 = pp.tile([P, NT * G], bf16, tag="B1")
            b1v = B1[:].rearrange("p (n g) -> p n g", g=G)

            s_sb = rp.tile([P, NT], f32, tag="s")
            iou_sb = rp.tile([P, NT], f32, tag="iou")
            ls = rp.tile([P, NT], f32, tag="ls")
            p_sb = rp.tile([P, NT], f32, tag="p")
            a_sb = rp.tile([P, NT], f32, tag="a")
            b_sb = rp.tile([P, NT], f32, tag="b")
            cnt = rp.tile([P, NT], f32, tag="cnt")
            W1 = rp.tile([P, NT], f32, tag="W1")
            W2 = rp.tile([P, NT], f32, tag="W2")

            conf_parts = rp.tile([P, nch * G], f32, tag="confp")
            mx_parts = rp.tile([P, nch * G], f32, tag="mxp")
            conf_row = rp.tile([P, G], f32, tag="confrow")
            rr_row = rp.tile([P, G], f32, tag="rrrow")
            tcol = rp.tile([G, P], f32, tag="tcol")
            red32 = rp.tile([G, 1], f32, tag="red32")

            # ---------- phase 0: per-row precompute ----------
            nc.sync.dma_start(s_sb[:], sco.ap())
            nc.sync.dma_start(iou_sb[:], iou.ap())
            lg_t = rt.tile([P, NT], f32, tag="rowtmp")
            nc.sync.dma_start(lg_t[:], lgt.ap())

            t0 = rt.tile([P, NT], f32, tag="rowtmp")
            nc.vector.tensor_scalar(t0[:], s_sb[:], 1e-38, None, Alu.max)
            nc.scalar.activation(ls[:], t0[:], Act.Ln)

            en = rt.tile([P, NT], f32, tag="rowtmp")
            nc.scalar.activation(en[:], lg_t[:], Act.Exp, scale=-1.0)
            nc.vector.tensor_scalar(en[:], en[:], 1.0, None, Alu.add)
            nc.vector.reciprocal(p_sb[:], en[:])
            nc.vector.tensor_scalar(
                p_sb[:], p_sb[:], CLAMP, 1.0 - CLAMP, Alu.max, Alu.min
            )
            om = rt.tile([P, NT], f32, tag="rowtmp")   # 1-p
            nc.vector.tensor_scalar(om[:], p_sb[:], -1.0, 1.0, Alu.mult, Alu.add)
            logp = rt.tile([P, NT], f32, tag="rowtmp")
            nc.scalar.activation(logp[:], p_sb[:], Act.Ln)
            log1mp = rt.tile([P, NT], f32, tag="rowtmp")
            nc.scalar.activation(log1mp[:], om[:], Act.Ln)
            q2 = rt.tile([P, NT], f32, tag="rowtmp")
            nc.scalar.activation(q2[:], om[:], Act.Square)
            nc.vector.tensor_tensor(a_sb[:], logp[:], q2[:], Alu.mult)
            p2 = rt.tile([P, NT], f32, tag="rowtmp")
            nc.scalar.activation(p2[:], p_sb[:], Act.Square)
            nc.vector.tensor_tensor(b_sb[:], log1mp[:], p2[:], Alu.mult)

            # ---------- phase A: conf, cnt, miou ----------
            for ci, (n0, k) in enumerate(chunks):
                it = sp.tile([P, CH * G], i32, tag="int")
                itv = it[:, : k * G].rearrange("p (n g) -> p n g", g=G)
                nc.sync.dma_start(itv, iib3[:, n0 : n0 + k, :])

                mk = sp.tile([P, CH * G], f32, tag="mask")
                mkv = mk[:, : k * G].rearrange("p (n g) -> p n g", g=G)
                nc.gpsimd.tensor_scalar(mkv, itv, 0, None, Alu.is_gt)
                nc.vector.tensor_reduce(
                    cnt[:, n0 : n0 + k], mkv, axis=AX.X, op=Alu.add
                )

                s_b = s_sb[:, n0 : n0 + k][:, :, None].to_broadcast([P, k, G])
                # am = (int > 0) * s  (overwrite mask stage after cnt)
                nc.vector.scalar_tensor_tensor(
                    mkv, itv, 0, s_b, Alu.is_gt, Alu.mult
                )
                nc.vector.tensor_reduce(
                    conf_parts[:, ci * G : (ci + 1) * G],
                    mk[:, : k * G].rearrange("p (n g) -> p g n", g=G),
                    axis=AX.X,
                    op=Alu.max,
                )
                iou_b = iou_sb[:, n0 : n0 + k][:, :, None].to_broadcast([P, k, G])
                nc.vector.scalar_tensor_tensor(
                    b1v[:, n0 : n0 + k, :], itv, 0, iou_b, Alu.is_gt, Alu.mult
                )

            # conf: combine chunks -> cross-partition -> cross-core
            conf128 = rt.tile([P, G], f32, tag="c128")
            nc.vector.tensor_reduce(
                conf128[:],
                conf_parts[:].rearrange("p (c g) -> p g c", g=G),
                axis=AX.X,
                op=Alu.max,
            )
            d_a = dp.tile([P, G], f32, tag="dA")
            nc.sync.dma_start(d_a[:], conf128[:])
            nc.sync.dma_start(tcol[:], d_a[:].rearrange("p g -> g p"))
            nc.vector.tensor_reduce(red32[:], tcol[:], axis=AX.X, op=Alu.max)
            cc_in = dp.tile([G, 1], f32, tag="ccin")
            cc_out = dp.tile([G, 1], f32, tag="ccout")
            nc.sync.dma_start(cc_in[:], red32[:])
            nc.gpsimd.collective_compute(
                "AllReduce",
                Alu.max,
                replica_groups=rg,
                ins=[cc_in[:]],
                outs=[cc_out[:]],
            )
            nc.sync.dma_start(
                conf_row[:], cc_out[:, 0][None, :].to_broadcast([P, G])
            )

            # ---------- phase B: raw = exp(conf*ls)*miou, mx ----------
            for ci, (n0, k) in enumerate(chunks):
                ts = sp.tile([P, CH * G], f32, tag="tstage")
                tsv = ts[:, : k * G].rearrange("p (n g) -> p n g", g=G)
                ls_b = ls[:, n0 : n0 + k][:, :, None].to_broadcast([P, k, G])
                cf_b = conf_row[:, None, :].to_broadcast([P, k, G])
                nc.gpsimd.tensor_tensor(tsv, ls_b, cf_b, Alu.mult)
                nc.scalar.activation(tsv, tsv, Act.Exp)
                nc.vector.tensor_tensor(
                    b1v[:, n0 : n0 + k, :], tsv, b1v[:, n0 : n0 + k, :], Alu.mult
                )
                nc.vector.tensor_reduce(
                    mx_parts[:, ci * G : (ci + 1) * G],
                    B1[:, n0 * G : (n0 + k) * G].rearrange("p (n g) -> p g n", g=G),
                    axis=AX.X,
                    op=Alu.max,
                )

            mx128 = rt.tile([P, G], f32, tag="c128")
            nc.vector.tensor_reduce(
                mx128[:],
                mx_parts[:].rearrange("p (c g) -> p g c", g=G),
                axis=AX.X,
                op=Alu.max,
            )
            d_b = dp.tile([P, G], f32, tag="dB")
            nc.sync.dma_start(d_b[:], mx128[:])
            nc.sync.dma_start(tcol[:], d_b[:].rearrange("p g -> g p"))
            nc.vector.tensor_reduce(red32[:], tcol[:], axis=AX.X, op=Alu.max)
            cc_in2 = dp.tile([G, 1], f32, tag="ccin2")
            cc_out2 = dp.tile([G, 1], f32, tag="ccout2")
            nc.sync.dma_start(cc_in2[:], red32[:])
            nc.gpsimd.collective_compute(
                "AllReduce",
                Alu.max,
                replica_groups=rg,
                ins=[cc_in2[:]],
                outs=[cc_out2[:]],
            )
            mx_row = rt.tile([P, G], f32, tag="c128")
            nc.sync.dma_start(
                mx_row[:], cc_out2[:, 0][None, :].to_broadcast([P, G])
            )
            nc.vector.tensor_scalar(mx_row[:], mx_row[:], CLAMP, None, Alu.add)
            nc.vector.reciprocal(rr_row[:], mx_row[:])

            # ---------- phase C: wc, W1, W2 ----------
            for ci, (n0, k) in enumerate(chunks):
                m2 = sp.tile([P, CH * G], f32, tag="mask2")
                m2v = m2[:, : k * G].rearrange("p (n g) -> p n g", g=G)
                nc.gpsimd.tensor_scalar(
                    m2v, b1v[:, n0 : n0 + k, :], 0.0, None, Alu.is_gt
                )
                us = sp.tile([P, CH * G], f32, tag="ustage")
                usv = us[:, : k * G].rearrange("p (n g) -> p n g", g=G)
                rr_b = rr_row[:, None, :].to_broadcast([P, k, G])
                nc.vector.scalar_tensor_tensor(
                    usv, b1v[:, n0 : n0 + k, :], CLAMP, rr_b, Alu.add, Alu.mult
                )
                nc.vector.tensor_scalar(
                    usv, usv, CLAMP, 1.0 - CLAMP, Alu.max, Alu.min
                )
                vs = sp.tile([P, CH * G], f32, tag="vstage")
                vsv = vs[:, : k * G].rearrange("p (n g) -> p n g", g=G)
                nc.vector.tensor_tensor(vsv, usv, m2v, Alu.mult)
                nc.vector.tensor_reduce(
                    W1[:, n0 : n0 + k], vsv, axis=AX.X, op=Alu.add
                )
                nc.scalar.activation(m2v, vsv, Act.Square)
                nc.vector.tensor_reduce(
                    W2[:, n0 : n0 + k], m2v, axis=AX.X, op=Alu.add
                )

            # ---------- epilogue ----------
            h1 = rt.tile([P, NT], f32, tag="rowtmp")
            nc.vector.scalar_tensor_tensor(
                h1[:], W1[:], -2.0, cnt[:], Alu.mult, Alu.add
            )
            bq = rt.tile([P, NT], f32, tag="rowtmp")
            nc.vector.tensor_tensor(bq[:], h1[:], W2[:], Alu.add)
            t1 = rt.tile([P, NT], f32, tag="rowtmp")
            nc.vector.tensor_tensor(t1[:], a_sb[:], W2[:], Alu.mult)
            t2 = rt.tile([P, NT], f32, tag="rowtmp")
            nc.vector.tensor_tensor(t2[:], b_sb[:], bq[:], Alu.mult)
            z = rt.tile([P, NT], f32, tag="rowtmp")
            nc.vector.tensor_scalar(z[:], cnt[:], 0.0, 3.0, Alu.is_equal, Alu.mult)
            t3 = rt.tile([P, NT], f32, tag="rowtmp")
            nc.vector.tensor_tensor(t3[:], z[:], b_sb[:], Alu.mult)
            acc = rt.tile([P, NT], f32, tag="rowtmp")
            nc.vector.tensor_tensor(acc[:], t1[:], t2[:], Alu.add)
            nc.vector.tensor_tensor(acc[:], acc[:], t3[:], Alu.add)
            rsum = rt.tile([P, 1], f32, tag="rsum")
            nc.vector.tensor_reduce(rsum[:], acc[:], axis=AX.X, op=Alu.add)
            nc.sync.dma_start(outp.ap(), rsum[:])

    nc.finalize()
    return nc


def _get_program():
    if "nc" not in _CACHE:
        _CACHE["nc"] = _build_program()
    return _CACHE["nc"]


def kernel(logits_pred, scores, IoUMap, is_in_boxes, num_pos_avg):
    lg = np.asarray(logits_pred, np.float32).reshape(-1)
    sc = np.asarray(scores, np.float32).reshape(-1)
    io = np.asarray(IoUMap, np.float32).reshape(-1)
    ib = np.asarray(is_in_boxes, np.int32).reshape(N, G)
    npa = float(np.asarray(num_pos_avg))

    lgp = np.full(NPAD, -30.0, np.float32)
    lgp[:N] = lg
    scp = np.full(NPAD, 0.5, np.float32)
    scp[:N] = sc
    iop = np.full(NPAD, 0.5, np.float32)
    iop[:N] = io
    ibp = np.zeros((NPAD, G), np.int32)
    ibp[:N] = ib

    in_maps = []
    for c in range(NCORES):
        s0 = c * NC_ROWS
        s1 = s0 + NC_ROWS
        in_maps.append(
            {
                "iib": np.ascontiguousarray(ibp[s0:s1]),
                "lgt": np.ascontiguousarray(lgp[s0:s1].reshape(NT, P).T),
                "sco": np.ascontiguousarray(scp[s0:s1].reshape(NT, P).T),
                "iou": np.ascontiguousarray(iop[s0:s1].reshape(NT, P).T),
            }
        )

    nc = _get_program()
    res = run_bass_kernel_spmd(nc, in_maps, core_ids=list(range(NCORES)))
    total = 0.0
    for r in res.results:
        total += float(np.asarray(r["outp"], np.float64).sum())
    return np.float32(-0.25 * total / npa)


if __name__ == "__main__":
    _get_program()
    print("program built ok")
